# revision 1
# baseline (speedup 1.0000x reference)
"""Multi-head self-attention (B=8, S=2048, H=256, NH=8, HD=32) on 8 TRN2 cores.

v2: data-parallel over batch (1 batch element/core). The baseline was
co-limited by ACT exp (~294us) and PE (~307us). Fixes:
  - exp split across BOTH ScalarE (exact exp) and VectorE (bit-hack exp:
    e^x ~= bitcast_bf16(int16(x*scale*128*log2e + (127*128 - c)))), each
    draining its own PSUM score half.
  - ctx matmuls batched W groups at a time so the PE's row<->col tiling
    mode alternation happens 1/W as often.
  - ctx accumulators live in ONE 4-bank PSUM tile: eviction + softmax
    normalization is a single full-height op per q-block.
  - rowsum reciprocal batched via DRAM round-trip into [128,32] layout.
  - out-projection col-tiled (same PE mode as ctx), writing into the ctx
    banks between q-blocks; output staged and DMA'd per q-block.
"""
import numpy as np
import ml_dtypes

import bass_rust
import concourse.bass as bass
import concourse.mybir as mybir
import concourse.tile as tile
from concourse.bass_utils import run_bass_kernel_spmd

BF16 = mybir.dt.bfloat16
F32 = mybir.dt.float32
I16 = mybir.dt.int16
NPBF16 = ml_dtypes.bfloat16

B, S, H = 8, 2048, 256
NH, HD = 8, 32
SCALE = 1.0 / float(np.sqrt(HD))
N_CORES = 8

LOG2E = 1.4426950408889634
# DVE bit-hack exp constants (c centers the mantissa-interp sawtooth; the
# +0.5 assumes truncation on float->int convert - set HACK_ROUND=True if the
# DVE rounds to nearest)
HACK_C = 5.5
HACK_ROUND = True
A_HACK = float(LOG2E * 128.0 * SCALE)
B_HACK = float(127.0 * 128.0 - HACK_C + (0.0 if HACK_ROUND else 0.5))

CTX_DELAY = 2        # groups between scores and their ctx consumption
# exp engine split: DVE takes DVE_SHARE of every 16 quarter-units
DVE_SHARE = 8

TRACE_OPTS = {}
LAST_RESULT = None


def _legalize_sync_waits(nc):
    """Split multi-wait sync_infos onto NoOp carriers (walrus allows 1/inst)."""
    n = 0
    for f in nc.m.functions:
        for bb in f.blocks:
            insts = bb.instructions
            i = 0
            while i < len(insts):
                inst = insts[i]
                si = inst.sync_info
                if si is not None and len(si.on_wait) > 1:
                    waits = list(si.on_wait)
                    carriers = []
                    for w in waits[:-1]:
                        carriers.append(
                            mybir.InstNoOp(
                                name=f"{inst.name}-w{n}",
                                sync_info=mybir.SyncInfo(on_wait=[w], on_update=[]),
                                bass_nofuse=True,
                                engine=inst.engine,
                            )
                        )
                        n += 1
                    inst.sync_info = bass_rust.SyncInfo(
                        on_wait=waits[-1:], on_update=list(si.on_update)
                    )
                    insts[i:i] = carriers
                    i += len(carriers)
                i += 1
    return n


def _build_nc(legalize=True):
    nc = bass.Bass()
    xt = nc.dram_tensor("xt", [128, 2 * S], BF16, kind="ExternalInput")
    wqk = nc.dram_tensor("wqk", [128, 2 * 512], BF16, kind="ExternalInput")
    bqk = nc.dram_tensor("bqk", [1, 512], BF16, kind="ExternalInput")
    bv = nc.dram_tensor("bv", [1, 264], BF16, kind="ExternalInput")
    wv = nc.dram_tensor("wv", [128, 2 * 264], BF16, kind="ExternalInput")
    wo = nc.dram_tensor("wo", [128, 4 * 256], BF16, kind="ExternalInput")
    ones = nc.dram_tensor("ones", [1, 512], BF16, kind="ExternalInput")
    out = nc.dram_tensor("out", [S, H], F32, kind="ExternalOutput")
    # rowsum gather / reciprocal round-trip scratch ([2,2048] <-> [128,32])
    rscr = nc.dram_tensor("rscr", [2, 2048], F32)
    rscr2 = nc.dram_tensor("rscr2", [2, 2048], F32)

    EXP = mybir.ActivationFunctionType.Exp

    with tile.TileContext(nc) as tc:
        with (
            tc.tile_pool(name="const", bufs=1) as const,
            tc.tile_pool(name="etp", bufs=12) as etp,
            tc.tile_pool(name="ctp", bufs=2) as ctp,
            tc.tile_pool(name="stp", bufs=2) as stp,
            tc.tile_pool(name="osb", bufs=2) as osb,
        ):
            xt_sb = const.tile([128, 2 * S], BF16, tag="xt")
            for ch in range(4):
                nc.sync.dma_start(
                    out=xt_sb[:, ch * 1024: ch * 1024 + 1024],
                    in_=xt[:, ch * 1024: ch * 1024 + 1024])
            wqk_sb = const.tile([128, 2 * 512], BF16, tag="wqk")
            nc.sync.dma_start(out=wqk_sb, in_=wqk[:, :])
            wv_sb = const.tile([128, 2 * 264], BF16, tag="wv")
            nc.sync.dma_start(out=wv_sb, in_=wv[:, :])
            wo_sb = const.tile([128, 4 * 256], BF16, tag="wo")
            nc.sync.dma_start(out=wo_sb, in_=wo[:, :])
            bqk_sb = const.tile([1, 512], BF16, tag="bqk")
            nc.sync.dma_start(out=bqk_sb, in_=bqk[:, :])
            bv_sb = const.tile([1, 264], BF16, tag="bv")
            nc.sync.dma_start(out=bv_sb, in_=bv[:, :])
            ones_sb = const.tile([1, 512], BF16, tag="ones")
            nc.sync.dma_start(out=ones_sb, in_=ones[:, :])

            qT_sb = const.tile([128, 2 * S], BF16, tag="qT")
            kT_sb = const.tile([128, 2 * S], BF16, tag="kT")
            v_sb = const.tile([128, 16 * 264], BF16, tag="v")

            # persistent PSUM: ctx accumulators (4 banks) + score ring (4)
            with (
                tc.tile_pool(name="cxp", bufs=1, space="PSUM") as cxp,
            ):
                ctx_ps = cxp.tile([128, 2048], F32, tag="ctx")
                # zero the never-matmul-written rows so the full-height
                # eviction reads finite values (persist across q-blocks)
                nc.vector.memset(ctx_ps[32:64, :], 0.0)
                nc.vector.memset(ctx_ps[96:128, :], 0.0)

                # ---- warmup + phase 1/2 run while input DMAs land ----
                warm_sb = const.tile([128, 512], BF16, tag="warm")
                nc.vector.memset(warm_sb, 0.0)
                for r in range(20):
                    nc.tensor.matmul(
                        out=ctx_ps[:, 0:512], lhsT=warm_sb[:, 0:128],
                        rhs=warm_sb[:, :], start=True, stop=True,
                    )

                # ---- phase 1: qT/kT [feature, s]; bias folded in via a
                #      rank-1 accumulate matmul; copy-evict split ACT/DVE ----
                p12 = tc.tile_pool(name="p12", bufs=2, space="PSUM")
                scp12 = p12.__enter__()
                for t in range(4):  # feature tiles: q0,q1,k0,k1
                    for nb in range(4):  # s blocks of 512
                        ps = scp12.tile([128, 1024], F32, tag="sc",
                                      name=f"p1_{t}_{nb}")
                        for ks in range(2):
                            nc.tensor.matmul(
                                out=ps[:, 0:512],
                                lhsT=wqk_sb[:, ks * 512 + t * 128: ks * 512 + t * 128 + 128],
                                rhs=xt_sb[:, ks * S + nb * 512: ks * S + nb * 512 + 512],
                                start=(ks == 0), stop=False,
                            )
                        nc.tensor.matmul(
                            out=ps[:, 0:512],
                            lhsT=bqk_sb[0:1, t * 128: t * 128 + 128],
                            rhs=ones_sb[0:1, :],
                            start=False, stop=True,
                        )
                        dst = (qT_sb if t < 2 else kT_sb)[
                            :, (t % 2) * S + nb * 512: (t % 2) * S + nb * 512 + 512
                        ]
                        if nb % 2 == 0:
                            nc.scalar.copy(out=dst, in_=ps[:, 0:512])
                        else:
                            nc.vector.tensor_copy(out=dst, in_=ps[:, 0:512])

                # ---- phase 2: v (padded 66-wide head-pair slots, ones col
                #      per head for rowsums; bias row plants the ones) ----
                for st in range(16):
                    ps = scp12.tile([128, 1024], F32, tag="sc", name=f"p2_{st}")
                    for ks in range(2):
                        nc.tensor.matmul(
                            out=ps[:, 0:264],
                            lhsT=xt_sb[:, ks * S + st * 128: ks * S + st * 128 + 128],
                            rhs=wv_sb[:, ks * 264: ks * 264 + 264],
                            start=(ks == 0), stop=False,
                        )
                    nc.tensor.matmul(
                        out=ps[:, 0:264],
                        lhsT=ones_sb[0:1, 0:128],
                        rhs=bv_sb[0:1, 0:264],
                        start=False, stop=True,
                    )
                    dst = v_sb[:, st * 264: st * 264 + 264]
                    if st % 2 == 0:
                        nc.scalar.copy(out=dst, in_=ps[:, 0:264])
                    else:
                        nc.vector.tensor_copy(out=dst, in_=ps[:, 0:264])

                p12.__exit__(None, None, None)

                # ---- phase 3: attention, q-blocks of 512 ----
                scp_cm = tc.tile_pool(name="scp", bufs=4, space="PSUM")
                scp = scp_cm.__enter__()
                prev = None  # (ctxT tile, qb) pending normalize+out-proj

                def emit_evict(qb, stg_t):
                    # evict ctx PSUM (full height; rowsums at rows 32/96) ...
                    nc.scalar.copy(out=stg_t, in_=ctx_ps)
                    # ... gather rowsums to DRAM rows
                    nc.sync.dma_start(out=rscr[0:1, :], in_=stg_t[32:33, :])
                    nc.sync.dma_start(out=rscr[1:2, :], in_=stg_t[96:97, :])

                def emit_recip(qb, rcb_t):
                    # reciprocal as [128,32], broadcast 1/rs across partitions
                    rsg = osb.tile([128, 32], F32, tag="rsg", name=f"rsg{qb}")
                    nc.sync.dma_start(out=rsg, in_=rscr[:, :])
                    nc.vector.reciprocal(out=rsg, in_=rsg)
                    nc.sync.dma_start(out=rscr2[:, :], in_=rsg)
                    nc.sync.dma_start(
                        out=rcb_t[0:64, :],
                        in_=rscr2[0:1, :].to_broadcast((64, 2048)),
                    )
                    nc.sync.dma_start(
                        out=rcb_t[64:128, :],
                        in_=rscr2[1:2, :].to_broadcast((64, 2048)),
                    )

                def emit_norm_tt(ctxT_t, stg_t, rcb_t, half):
                    h0 = half * 1024
                    nc.vector.tensor_mul(
                        out=ctxT_t[:, h0:h0 + 1024], in0=stg_t[:, h0:h0 + 1024],
                        in1=rcb_t[:, h0:h0 + 1024])

                def emit_outproj(qb, ctxT_t, osb_t):
                    # col-tiled (same PE mode as ctx); writes into ctx banks
                    for st in range(4):
                        for pair in range(4):
                            for cg in range(2):
                                nc.tensor.matmul(
                                    out=ctx_ps[64 * cg: 64 * cg + 64,
                                               st * 512: st * 512 + 256],
                                    lhsT=ctxT_t[:, pair * 512 + st * 128 + 64 * cg:
                                                pair * 512 + st * 128 + 64 * cg + 64],
                                    rhs=wo_sb[:, pair * 256: pair * 256 + 256],
                                    start=(pair == 0), stop=(pair == 3),
                                    tile_position=(0, 64 * cg),
                                    skip_group_check=True,
                                )
                        nc.scalar.copy(
                            out=osb_t[:, st * 256: st * 256 + 256],
                            in_=ctx_ps[:, st * 512: st * 512 + 256],
                        )
                        nc.sync.dma_start(
                            out=out[qb * 512 + st * 128: qb * 512 + st * 128 + 128, :],
                            in_=osb_t[:, st * 256: st * 256 + 256],
                        )

                def emit_ctx_batch(batch):
                    for (g, kt, eT) in batch:
                        for pi in range(2):
                            pair = g * 2 + pi
                            vc = kt * 264 + pair * 66
                            nc.tensor.matmul(
                                out=ctx_ps[0:33, pair * 512: pair * 512 + 512],
                                lhsT=v_sb[:, vc: vc + 33],
                                rhs=eT[:, (2 * pi) * 512: (2 * pi) * 512 + 512],
                                start=(kt == 0), stop=(kt == 15),
                                tile_position=(0, 0), skip_group_check=True,
                            )
                            nc.tensor.matmul(
                                out=ctx_ps[64:97, pair * 512: pair * 512 + 512],
                                lhsT=v_sb[:, vc + 33: vc + 66],
                                rhs=eT[:, (2 * pi + 1) * 512: (2 * pi + 1) * 512 + 512],
                                start=(kt == 0), stop=(kt == 15),
                                tile_position=(0, 64), skip_group_check=True,
                            )

                for qb in range(4):
                    if prev is not None:
                        pqb, pctxT, pstg, prcb, po = prev
                        emit_evict(pqb, pstg)
                        emit_recip(pqb, prcb)

                    pending = []   # filled-score groups awaiting ctx
                    done_ctx = 0
                    unit_idx = 0
                    groups = [(g, kt) for kt in range(16) for g in range(2)]
                    for bi, (g, kt) in enumerate(groups):
                        eT = etp.tile([128, 2048], BF16, tag="eT",
                                      name=f"eT_{qb}_{g}_{kt}")
                        for i in range(4):
                            sc = scp.tile([128, 512], F32, tag="sc",
                                          name=f"sc_{qb}_{kt}_{g}_{i}")
                            nc.tensor.matmul(
                                out=sc,
                                lhsT=kT_sb[32 * i: 32 * i + 32,
                                           g * S + kt * 128: g * S + kt * 128 + 128],
                                rhs=qT_sb[32 * i: 32 * i + 32,
                                          g * S + qb * 512: g * S + qb * 512 + 512],
                                start=True, stop=True,
                                tile_position=(32 * i, 0),
                            )
                            eT_q = eT[:, i * 512: i * 512 + 512]
                            use_dve = (unit_idx * DVE_SHARE) % 16 < DVE_SHARE
                            if not use_dve:
                                nc.scalar.activation(
                                    out=eT_q, in_=sc, func=EXP, scale=SCALE,
                                )
                            else:
                                nc.vector.tensor_scalar(
                                    out=eT_q.bitcast(I16), in0=sc,
                                    scalar1=A_HACK, scalar2=B_HACK,
                                    op0=mybir.AluOpType.mult,
                                    op1=mybir.AluOpType.add,
                                )
                            unit_idx += 1
                        pending.append([(g, kt, eT)])

                        if prev is not None:
                            pqb, pctxT, pstg, prcb, po = prev
                            if bi == 0:
                                emit_norm_tt(pctxT, pstg, prcb, 0)
                                emit_norm_tt(pctxT, pstg, prcb, 1)
                            elif bi == 1:
                                emit_outproj(pqb, pctxT, po)
                                prev = None
                        if bi >= CTX_DELAY:
                            emit_ctx_batch(pending[done_ctx])
                            done_ctx += 1
                    while done_ctx < len(pending):
                        emit_ctx_batch(pending[done_ctx])
                        done_ctx += 1

                    ctxT_t = ctp.tile([128, 2048], BF16, tag="ctxT",
                                      name=f"ctxT{qb}")
                    po_t = osb.tile([128, 1024], F32, tag="ot", name=f"ot{qb}")
                    stg_t = stp.tile([128, 2048], F32, tag="stg",
                                     name=f"stg{qb}")
                    rcb_t = stp.tile([128, 2048], F32, tag="rcb",
                                     name=f"rcb{qb}")
                    prev = (qb, ctxT_t, stg_t, rcb_t, po_t)

                # tail: last q-block normalize + out-proj
                pqb, pctxT, pstg, prcb, po = prev
                emit_evict(pqb, pstg)
                emit_recip(pqb, prcb)
                emit_norm_tt(pctxT, pstg, prcb, 0)
                emit_norm_tt(pctxT, pstg, prcb, 1)
                emit_outproj(pqb, pctxT, po)
                scp_cm.__exit__(None, None, None)
    if legalize:
        _legalize_sync_waits(nc)
    return nc


_NC_CACHE = None


def _get_nc():
    global _NC_CACHE
    if _NC_CACHE is None:
        _NC_CACHE = _build_nc()
    return _NC_CACHE


def _ks_layout(a, nk, cols):
    """[nk*128, cols] -> [128, nk*cols] with [p, k*cols+c] = a[k*128+p, c]."""
    return np.ascontiguousarray(
        a.reshape(nk, 128, cols).transpose(1, 0, 2).reshape(128, nk * cols)
    )


def _prep_in_maps(x, w_qkv, b_qkv, w_out, b_out):
    x = np.asarray(x, dtype=np.float32)
    w_qkv = np.asarray(w_qkv, dtype=np.float32)
    b_qkv = np.asarray(b_qkv, dtype=np.float32)
    w_out = np.asarray(w_out, dtype=np.float32)
    b_out = np.asarray(b_out, dtype=np.float32)

    wqk_l = _ks_layout(w_qkv[:, : 2 * H], 2, 512).astype(NPBF16)

    # v weights: head-pair slots of 66: [v_2p |1| v_2p+1 |1] (ones via bias row)
    wpad = np.zeros((H, 264), np.float32)
    bvr = np.zeros((1, 264), np.float32)
    for h in range(NH):
        c0 = (h // 2) * 66 + (h % 2) * 33
        wpad[:, c0: c0 + 32] = w_qkv[:, 2 * H + h * HD: 2 * H + (h + 1) * HD]
        bvr[0, c0: c0 + 32] = b_qkv[2 * H + h * HD: 2 * H + (h + 1) * HD]
        bvr[0, c0 + 32] = 1.0
    wv_l = _ks_layout(wpad, 2, 264).astype(NPBF16)

    # w_out rows permuted into ctxT slot layout: per pair-block of 128 rows:
    # [head 2p (32) | b_out row (pair 0 only) | 31 zeros | head 2p+1 (32) | 32 zeros]
    wo_perm = np.zeros((512, H), np.float32)
    for pair in range(4):
        r0 = pair * 128
        wo_perm[r0: r0 + 32, :] = w_out[(2 * pair) * HD: (2 * pair + 1) * HD, :]
        wo_perm[r0 + 64: r0 + 96, :] = w_out[(2 * pair + 1) * HD: (2 * pair + 2) * HD, :]
    wo_perm[32, :] = b_out  # ctxT row 32 of pair 0 is rs*1/rs = 1
    wo_l = _ks_layout(wo_perm, 4, 256).astype(NPBF16)

    shared = {
        "wqk": wqk_l,
        "wv": wv_l,
        "bqk": b_qkv[: 2 * H].reshape(1, 512).astype(NPBF16),
        "bv": bvr.astype(NPBF16),
        "wo": wo_l,
        "ones": np.ones((1, 512), NPBF16),
    }
    in_maps = []
    for b in range(B):
        xtm = _ks_layout(np.ascontiguousarray(x[b].T), 2, S).astype(NPBF16)
        in_maps.append({"xt": xtm, **shared})
    return in_maps


def kernel(x, w_qkv, b_qkv, w_out, b_out):
    in_maps = _prep_in_maps(x, w_qkv, b_qkv, w_out, b_out)
    nc = _get_nc()
    res = run_bass_kernel_spmd(nc, in_maps, list(range(N_CORES)), **TRACE_OPTS)
    global LAST_RESULT
    LAST_RESULT = res
    return np.stack([res.results[b]["out"] for b in range(B)], axis=0)

